# revision 11
# baseline (speedup 1.0000x reference)
"""Bhattacharyya coefficient kernel for Trainium2 (8 NeuronCores, SPMD).

out[n,0,i,j] = (1/k^2) * sum_{c,p,q} w[c] * sqrt(x[n,c,i+p,j+q] * z[n,c,p,q])

Data-parallel over batch: 2 samples per core. Per sample:
  1. ACT: sx = sqrt(x) (bf16), szw = w/k^2 * sqrt(z) (bf16).
  2. TensorE, *column-tiled*: each 512-col block of the correlation
     plane[t, y] = sum_c szw[c, t] * sx[c, y]   (t = 8p+q, 64 taps)
     is computed as two concurrent 64x256 matmuls in PE column groups
     0-1 / 2-3 (tile_position (0,0) and (0,64)), so PSUM fills all 128
     partitions: psum[64h+t, c] = plane[t, 512b + 256h + c].  This
     halves both the PE wall time and the DVE eviction cycles.
  3. DVE casts PSUM -> fp8-e4m3 plane tiles [128, 256*nblk].
  4. Dump to flat DRAM scratch (the h-interleave unrolls with a 3-dim
     AP, 2 DMAs per tensor) and gather back tap-aligned:
       A[t, u] = plane[t, u + 63*(t>>3) + (t&7)]
     making the tap-sum a pure partition reduction.  Sample 0's scratch
     DMAs ride the SWDGE ring (shares HBM with the x stream), sample
     1's late ones ride the Sync ring, which has drained by then.
  5. TensorE ones-matmul per gather chunk, col-tiled so chunk ch lands
     on PSUM partition 32*ch; DVE copies rows into obuf[32*ch]; three
     row-range DMAs ship out[i, j] = o[63 i + j].

Engines execute in rough emission order, so stage-2 compute of a sample
is emitted interleaved into the next sample's stage-1 at matching
readiness (a mis-ordered emission measurably serializes the kernel).

fp8 plane: values are O(1)..4 and the tap sum averages 64 independent
quantization errors -> absmax rel err ~1.1e-2 < the 2e-2 gate, while
halving scratch DRAM traffic.
"""

import numpy as np

import concourse.bacc as bacc
import concourse.bass as bass
import concourse.mybir as mybir
from concourse import tile
from concourse.bass_utils import run_bass_kernel_spmd

N, C, KS, MS = 16, 256, 8, 63
MO = MS - KS + 1            # 56
F = MS * MS                 # 3969
NCORES = 8
SPC = N // NCORES           # samples per core
BLK = 512
HB = 256                    # half-block (one PE column-group's share)
W = (MO - 1) * MS + MO      # 3521: last needed o index is 63*55+55
AF = mybir.ActivationFunctionType
f32 = mybir.dt.float32
bf16 = mybir.dt.bfloat16
fp8 = mybir.dt.float8e4

# x pieces (start block, n blocks) per sample: sample 1 finishes with
# two tiny pieces to shorten the end-of-stream -> sqrt -> matmul tail.
PIECES = {
    0: [(0, 2), (2, 2), (4, 2), (6, 2)],
    1: [(0, 2), (2, 2), (4, 2), (6, 1), (7, 1)],
}

# DRAM scratch tensors (plane col ranges) and gather chunks:
#   sc0 [0, 2560)    <- plane tile pl0 (blocks 0-4)
#   sc1a [2048, 3584) <- pl1 (blocks 4-6)
#   sc1b [2560, 3969) <- pl1 cols 256.. (blocks 5-6) + pl7 (block 7)
SC0, SC1A, SC1B = (0, 2560), (2048, 3584), (2560, 3969)
# chunks: (u0, ulen, sc, off, psum2 row): off = u0 - sc_start
CHUNKS = [
    (0, 1008, 0, 0, 0),
    (1008, 1008, 0, 1008, 32),
    (2016, 96, 0, 2016, 64),        # ch2a: tail of sc0's reach
    (2112, 912, 1, 64, 64),         # ch2b: rest of rows 32-47
    (3024, 497, 2, 464, 96),
]
SC_PIT = [2560, 1536, 1409]

_CACHE = {}


def _build():
    nc = bacc.Bacc("TRN2", target_bir_lowering=False, debug=False)
    z_in = nc.declare_dram_parameter("z", [SPC, C, KS, KS], f32, isOutput=False)
    x_in = nc.declare_dram_parameter("x", [SPC, C, MS, MS], f32, isOutput=False)
    w_in = nc.declare_dram_parameter("w", [C], f32, isOutput=False)
    out = nc.declare_dram_parameter("out", [SPC, 1, MO, MO], f32, isOutput=True)

    scs = [
        [nc.dram_tensor(f"sc{ci}_{s}", [64, SC_PIT[ci]], fp8) for ci in range(3)]
        for s in range(SPC)
    ]

    xflat = x_in.rearrange("s (k c) h w -> s k c (h w)", c=128)  # [SPC,2,128,F]

    with tile.TileContext(nc) as tc:
        with (
            tc.tile_pool(name="xstage", bufs=18) as xstage,
            tc.tile_pool(name="sxq", bufs=8) as sxq,
            tc.tile_pool(name="zpool", bufs=2) as zpool,
            tc.tile_pool(name="plane", bufs=2) as plane,
            tc.tile_pool(name="gath", bufs=5) as gath,
            tc.tile_pool(name="opool", bufs=1) as opool,
            tc.tile_pool(name="psum", bufs=3, space="PSUM") as psum,
            tc.tile_pool(name="psum7", bufs=1, space="PSUM") as psum7,
            tc.tile_pool(name="psum2", bufs=2, space="PSUM") as psum2,
        ):
            # ---- Sync ring: first x piece, tiny z/w loads, rest of x.
            xst = {}

            def load_piece(s, k, pi):
                b0, nbk = PIECES[s][pi]
                lo = b0 * BLK
                ln = min(nbk * BLK, F - lo)
                t = xstage.tile([128, 2 * BLK], f32, tag="xst",
                                name=f"xst{s}{k}{pi}")
                nc.sync.dma_start(t[:, :ln], xflat[s, k, :, lo : lo + ln])
                xst[(s, k, pi)] = t

            load_piece(0, 0, 0)
            wt = zpool.tile([128, 2], f32, name="wt")
            nc.sync.dma_start(wt[:], w_in.rearrange("(k c) -> c k", c=128))
            zts = []
            for s in range(SPC):
                zt = zpool.tile([128, 2, KS * KS], f32, tag="zt", name=f"zt{s}")
                nc.sync.dma_start(
                    zt[:], z_in[s].rearrange("(k c) p q -> c k (p q)", c=128)
                )
                zts.append(zt)
            load_piece(0, 1, 0)
            for s in range(SPC):
                for pi in range(len(PIECES[s])):
                    for k in range(2):
                        if (s, pi) != (0, 0):
                            load_piece(s, k, pi)

            ones = opool.tile([64, 1], fp8, name="ones")
            nc.gpsimd.memset(ones[:], 1.0)
            w64 = zpool.tile([128, 2], f32, name="w64")
            nc.vector.tensor_scalar_mul(w64[:], wt[:], 1.0 / (KS * KS))

            obufs, planes, ats = {}, {}, {}

            def eng(s):
                # sample 0's scratch DMAs share the stream (SWDGE ring);
                # sample 1's run when the Sync ring has drained.
                return nc.gpsimd if s == 0 else nc.sync

            def dump_sc0(s):
                pl0 = planes[s][0]
                for h in range(2):
                    dst = bass.AP(scs[s][0][:].tensor, h * HB,
                                  [[2560, 64], [BLK, 5], [1, HB]])
                    nc.gpsimd.dma_start(dst, pl0[64 * h : 64 * h + 64, :])
                gather(s, 0)
                gather(s, 1)
                gather(s, 2)

            def dump_sc1a(s):
                pl1 = planes[s][1]
                for h in range(2):
                    dst = bass.AP(scs[s][1][:].tensor, h * HB,
                                  [[1536, 64], [BLK, 3], [1, HB]])
                    eng(s).dma_start(dst, pl1[64 * h : 64 * h + 64, :])
                gather(s, 3)

            def dump_sc1b(s):
                pl1, pl7 = planes[s][1], planes[s][2]
                for h in range(2):
                    dst = bass.AP(scs[s][2][:].tensor, h * HB,
                                  [[1409, 64], [BLK, 2], [1, HB]])
                    eng(s).dma_start(dst, pl1[64 * h : 64 * h + 64, HB : 3 * HB])
                dst = bass.AP(scs[s][2][:].tensor, 1024, [[1409, 64], [1, 385]])
                eng(s).dma_start(dst, pl7[:, 0:385])
                gather(s, 4)

            def gather(s, ch):
                u0, ulen, sc, off, _ = CHUNKS[ch]
                pit = SC_PIT[sc]
                a = gath.tile([64, 1008], fp8, tag="a", name=f"a{s}_{ch}")
                src = bass.AP(
                    scs[s][sc][:].tensor, off,
                    [[8 * pit + MS, 8], [pit + 1, 8], [1, ulen]],
                )
                eng(s).dma_start(a[:, :ulen], src)
                ats[(s, ch)] = a

            def compute(s, ch):
                """tap-reduce matmuls -> psum2 row, DVE copy into obuf."""
                u0, ulen, sc, off, row = CHUNKS[ch]
                a = ats[(s, ch)]
                ps2 = psum2s[s]
                cbase = 0 if ch != 3 else 96
                m0 = 0
                while m0 < ulen:
                    nb = min(BLK - (cbase + m0) % BLK, ulen - m0)
                    nc.tensor.matmul(
                        ps2[row : row + 1, cbase + m0 : cbase + m0 + nb],
                        ones[:],
                        a[:, m0 : m0 + nb],
                        start=True,
                        stop=True,
                        tile_position=(0, row),
                    )
                    m0 += nb
                nc.vector.tensor_copy(
                    obufs[s][row : row + 1, cbase : cbase + ulen], ps2[row : row + 1, cbase : cbase + ulen]
                )
                # ship finished output row ranges (flat offset -> row 32g)
                ob_t = obufs[s][:].tensor
                if ch == 1:      # rows 0-31 from partitions 0, 32
                    osrc = bass.AP(ob_t, 0, [[32 * 1024, 2], [MS, 16], [1, MO]])
                    eng(s).dma_start(out[s, 0, 0:32].unsqueeze(0), osrc)
                elif ch == 3:    # rows 32-47 from partition 64
                    osrc = obufs[s][64:65, 0 : 16 * MS].rearrange(
                        "p (i j) -> p i j", i=16
                    )[:, :, 0:MO]
                    eng(s).dma_start(out[s, 0, 32:48].unsqueeze(0), osrc)
                elif ch == 4:    # rows 48-55 from partition 96
                    osrc = obufs[s][96:97, 0 : 8 * MS].rearrange(
                        "p (i j) -> p i j", i=8
                    )[:, :, 0:MO]
                    eng(s).dma_start(out[s, 0, 48:MO].unsqueeze(0), osrc)

            # stage-2 compute hooks: (sample, end-of-piece-idx) -> chunks
            hooks = {
                (0, 3): [(0, 0), (0, 1), (0, 2)],
                (1, 0): [(0, 3)],
                (1, 1): [(0, 4)],
                (1, 3): [(1, 0), (1, 1), (1, 2)],
                (1, 4): [(1, 3), (1, 4)],
            }

            psum2s = {}
            for s in range(SPC):
                obufs[s] = opool.tile([128, 1024], f32, tag=f"ob{s}",
                                      name=f"obuf{s}")
                psum2s[s] = psum2.tile([128, 2 * BLK], f32, tag="ps2",
                                       name=f"ps2_{s}")
                zsq = zpool.tile([128, 2, KS * KS], f32, tag="zsq",
                                 name=f"zsq{s}")
                szw = zpool.tile([128, 2, KS * KS], bf16, tag="szw",
                                 name=f"szw{s}")
                for kk in range(2):
                    nc.scalar.activation(zsq[:, kk, :], zts[s][:, kk, :],
                                         AF.Sqrt)
                    nc.vector.tensor_scalar_mul(
                        szw[:, kk, :], zsq[:, kk, :], w64[:, kk : kk + 1]
                    )
                planes[s] = [
                    plane.tile([128, 5 * HB], fp8, tag="pl0", name=f"pl0_{s}"),
                    plane.tile([128, 3 * HB], fp8, tag="pl1", name=f"pl1_{s}"),
                    plane.tile([64, 385], fp8, tag="pl7", name=f"pl7_{s}"),
                ]

                for pi, (b0, nbk) in enumerate(PIECES[s]):
                    lo = b0 * BLK
                    ln = min(nbk * BLK, F - lo)
                    sxp = {}
                    for k in range(2):
                        t = sxq.tile([128, 2 * BLK], bf16, tag="sxp",
                                     name=f"sxp{s}{k}{pi}")
                        nc.scalar.activation(
                            t[:, :ln], xst[(s, k, pi)][:, :ln], AF.Sqrt
                        )
                        sxp[k] = t
                    for j in range(nbk):
                        b = b0 + j
                        if b < 7:
                            # col-tiled pair: halves -> psum partitions
                            # [0:64] and [64:128]
                            ps = psum.tile([128, HB], f32, tag="ps",
                                           name=f"ps_{s}_{b}")
                            for k in range(2):
                                for h in range(2):
                                    nc.tensor.matmul(
                                        ps[64 * h : 64 * h + 64, :],
                                        szw[:, k, :],
                                        sxp[k][:, j * BLK + h * HB
                                               : j * BLK + h * HB + HB],
                                        start=(k == 0),
                                        stop=(k == 1),
                                        tile_position=(0, 64 * h),
                                    )
                            # fp8 evictions into the plane tiles
                            if b <= 4:
                                nc.vector.tensor_copy(
                                    planes[s][0][:, HB * b : HB * b + HB],
                                    ps[:],
                                )
                            if 4 <= b <= 6:
                                nc.vector.tensor_copy(
                                    planes[s][1][:, HB * (b - 4)
                                                 : HB * (b - 4) + HB],
                                    ps[:],
                                )
                        else:
                            ps7 = psum7.tile([64, BLK], f32, tag="ps7",
                                             name=f"ps7_{s}")
                            for k in range(2):
                                nc.tensor.matmul(
                                    ps7[:, :385],
                                    szw[:, k, :],
                                    sxp[k][:, j * BLK : j * BLK + 385],
                                    start=(k == 0),
                                    stop=(k == 1),
                                )
                            nc.vector.tensor_copy(planes[s][2][:, 0:385],
                                                  ps7[:, :385])
                        if b == 4:
                            dump_sc0(s)
                        elif b == 6:
                            dump_sc1a(s)
                        elif b == 7:
                            dump_sc1b(s)
                    for item in hooks.get((s, pi), ()):
                        compute(*item)

    nc.compile()
    return nc


def _get_nc():
    if "nc" not in _CACHE:
        _CACHE["nc"] = _build()
    return _CACHE["nc"]


def _run(z, x, weights, **runkw):
    z = np.ascontiguousarray(np.asarray(z), dtype=np.float32)
    x = np.ascontiguousarray(np.asarray(x), dtype=np.float32)
    w = np.ascontiguousarray(np.asarray(weights), dtype=np.float32).reshape(C)
    in_maps = []
    for i in range(NCORES):
        lo, hi = i * SPC, (i + 1) * SPC
        in_maps.append({"z": z[lo:hi], "x": x[lo:hi], "w": w})
    nc = _get_nc()
    try:
        res = run_bass_kernel_spmd(
            nc, in_maps, core_ids=list(range(NCORES)), **runkw
        )
    except Exception:
        # transient device errors have been observed to succeed on retry
        res = run_bass_kernel_spmd(
            nc, in_maps, core_ids=list(range(NCORES)), **runkw
        )
    full = np.concatenate([res.results[i]["out"] for i in range(NCORES)], axis=0)
    return full, res


def kernel(z, x, weights):
    full, _ = _run(z, x, weights)
    return full


# revision 12
# speedup vs baseline: 1.1003x; 1.1003x over previous
"""Bhattacharyya coefficient kernel for Trainium2 (8 NeuronCores, SPMD).

out[n,0,i,j] = (1/k^2) * sum_{c,p,q} w[c] * sqrt(x[n,c,i+p,j+q] * z[n,c,p,q])

Data-parallel over batch: 2 samples per core. Per sample:
  1. ACT: sx = sqrt(x) (bf16), szw = w/k^2 * sqrt(z) (bf16).
  2. TensorE, column-tiled: each 512-col block of the correlation
     plane[t, y] = sum_c szw[c, t] * sx[c, y]   (t = 8p+q, 64 taps)
     runs as two concurrent 64x256 matmuls in PE column groups 0-1/2-3,
     filling all 128 PSUM partitions: psum[64h+t, c] =
     plane[t, 512b + 256h + c].  Halves PE wall time and DVE eviction
     cycles vs a single 64x512 matmul.
  3. DVE casts PSUM -> fp8-e4m3 plane tiles [128, 256*nblk].
  4. Dump to flat DRAM scratch (the h-interleave unrolls with a 3-dim
     AP: 2 DMAs per tensor) and gather back tap-aligned
       A[t, u] = plane[t, u + 63*(t>>3) + (t&7)]
     -- all on the SWDGE ring, overlapping the x stream (queues share
     SDMA engines round-robin), with per-tensor exact dependencies.
  5. After ALL stage-1 work: per chunk a ones-matmul (col-tiled so chunk
     ch lands on PSUM partition 32ch of one [128,1024] bank pair), ONE
     [128,1024] DVE eviction per sample, three row-range out DMAs on
     the by-then-drained Sync ring.

Stage-2 compute is emitted strictly after both samples' stage-1:
engines execute in near-emission order, and any interleaving that does
not exactly match data readiness serializes the pipeline (measured
+13us).  The x stream owns the Sync ring alone so it issues
back-to-back; sample 1 ends with two tiny pieces to shorten the
end-of-stream -> sqrt -> matmul -> dump tail.

fp8 plane: values are O(1)..4 and the tap sum averages 64 independent
quantization errors -> absmax rel err ~1.1e-2 < the 2e-2 gate, while
halving scratch DRAM traffic.
"""

import numpy as np

import concourse.bacc as bacc
import concourse.bass as bass
import concourse.mybir as mybir
from concourse import tile
from concourse.bass_utils import run_bass_kernel_spmd

N, C, KS, MS = 16, 256, 8, 63
MO = MS - KS + 1            # 56
F = MS * MS                 # 3969
NCORES = 8
SPC = N // NCORES           # samples per core
BLK = 512
HB = 256                    # half-block (one PE column-group's share)
W = (MO - 1) * MS + MO      # 3521: last needed o index is 63*55+55
AF = mybir.ActivationFunctionType
f32 = mybir.dt.float32
bf16 = mybir.dt.bfloat16
fp8 = mybir.dt.float8e4

# x pieces (start block, n blocks); the small trailing pieces keep the
# final sqrt off the critical tail.
PIECES = [(0, 4), (4, 2), (6, 1), (7, 1)]

# DRAM scratch tensors (plane col ranges):
#   sc0 [0, 2560)     <- plane tile pl0 (blocks 0-4)
#   sc1a [2048, 3584) <- pl1 (blocks 4-6)
#   sc1b [2560, 3969) <- pl1 cols 256.. (blocks 5-6) + pl7 (block 7)
# chunks: (u0, ulen, sc, off, psum2 row)
CHUNKS = [
    (0, 1008, 0, 0, 0),
    (1008, 1008, 0, 1008, 32),
    (2016, 96, 0, 2016, 64),        # ch2a: tail of sc0's reach
    (2112, 912, 1, 64, 64),         # ch2b: rest of rows 32-47
    (3024, 497, 2, 464, 96),
]
SC_PIT = [2560, 1536, 1409]

_CACHE = {}


def _build():
    nc = bacc.Bacc("TRN2", target_bir_lowering=False, debug=False)
    z_in = nc.declare_dram_parameter("z", [SPC, C, KS, KS], f32, isOutput=False)
    x_in = nc.declare_dram_parameter("x", [SPC, C, MS, MS], f32, isOutput=False)
    w_in = nc.declare_dram_parameter("w", [C], f32, isOutput=False)
    out = nc.declare_dram_parameter("out", [SPC, 1, MO, MO], f32, isOutput=True)

    scs = [
        [nc.dram_tensor(f"sc{ci}_{s}", [64, SC_PIT[ci]], fp8) for ci in range(3)]
        for s in range(SPC)
    ]

    xflat = x_in.rearrange("s (k c) h w -> s k c (h w)", c=128)  # [SPC,2,128,F]

    with tile.TileContext(nc) as tc:
        with (
            tc.tile_pool(name="xstage", bufs=4) as xstage,
            tc.tile_pool(name="sxq", bufs=4) as sxq,
            tc.tile_pool(name="zpool", bufs=2) as zpool,
            tc.tile_pool(name="plane", bufs=2) as plane,
            tc.tile_pool(name="gath", bufs=10) as gath,
            tc.tile_pool(name="opool", bufs=1) as opool,
            tc.tile_pool(name="psum", bufs=3, space="PSUM") as psum,
            tc.tile_pool(name="psum7", bufs=1, space="PSUM") as psum7,
            tc.tile_pool(name="psum2", bufs=2, space="PSUM") as psum2,
        ):
            # ---- Sync ring: first x piece, tiny z/w loads, rest of x.
            xst = {}

            def load_piece(s, k, pi):
                b0, nbk = PIECES[pi]
                lo = b0 * BLK
                ln = min(nbk * BLK, F - lo)
                t = xstage.tile([128, nbk * BLK], f32, tag=f"xst{pi}",
                                name=f"xst{s}{k}{pi}")
                nc.sync.dma_start(t[:, :ln], xflat[s, k, :, lo : lo + ln])
                xst[(s, k, pi)] = t

            load_piece(0, 0, 0)
            wt = zpool.tile([128, 2], f32, name="wt")
            nc.sync.dma_start(wt[:], w_in.rearrange("(k c) -> c k", c=128))
            zts = []
            for s in range(SPC):
                zt = zpool.tile([128, 2, KS * KS], f32, tag="zt", name=f"zt{s}")
                nc.sync.dma_start(
                    zt[:], z_in[s].rearrange("(k c) p q -> c k (p q)", c=128)
                )
                zts.append(zt)
            load_piece(0, 1, 0)
            for s in range(SPC):
                for pi in range(len(PIECES)):
                    for k in range(2):
                        if (s, pi) != (0, 0):
                            load_piece(s, k, pi)

            ones = opool.tile([64, 1], fp8, name="ones")
            nc.gpsimd.memset(ones[:], 1.0)
            w64 = zpool.tile([128, 2], f32, name="w64")
            nc.vector.tensor_scalar_mul(w64[:], wt[:], 1.0 / (KS * KS))

            obufs, planes, ats, psum2s = {}, {}, {}, {}

            def dump_sc0(s):
                pl0 = planes[s][0]
                for h in range(2):
                    dst = bass.AP(scs[s][0][:].tensor, h * HB,
                                  [[2560, 64], [BLK, 5], [1, HB]])
                    nc.gpsimd.dma_start(dst, pl0[64 * h : 64 * h + 64, :])
                gather(s, 0)
                gather(s, 1)
                gather(s, 2)

            def dump_sc1a(s):
                pl1 = planes[s][1]
                for h in range(2):
                    dst = bass.AP(scs[s][1][:].tensor, h * HB,
                                  [[1536, 64], [BLK, 3], [1, HB]])
                    nc.gpsimd.dma_start(dst, pl1[64 * h : 64 * h + 64, :])
                gather(s, 3)

            def dump_sc1b(s):
                pl1, pl7 = planes[s][1], planes[s][2]
                for h in range(2):
                    dst = bass.AP(scs[s][2][:].tensor, h * HB,
                                  [[1409, 64], [BLK, 2], [1, HB]])
                    nc.gpsimd.dma_start(dst, pl1[64 * h : 64 * h + 64,
                                                 HB : 3 * HB])
                dst = bass.AP(scs[s][2][:].tensor, 1024, [[1409, 64], [1, 385]])
                nc.gpsimd.dma_start(dst, pl7[:, 0:385])
                gather(s, 4)

            def gather(s, ch):
                u0, ulen, sc, off, _ = CHUNKS[ch]
                pit = SC_PIT[sc]
                a = gath.tile([64, 1008], fp8, tag="a", name=f"a{s}_{ch}")
                src = bass.AP(
                    scs[s][sc][:].tensor, off,
                    [[8 * pit + MS, 8], [pit + 1, 8], [1, ulen]],
                )
                nc.gpsimd.dma_start(a[:, :ulen], src)
                ats[(s, ch)] = a

            # ---------------- stage 1 ----------------
            for s in range(SPC):
                obufs[s] = opool.tile([128, 1024], f32, tag=f"ob{s}",
                                      name=f"obuf{s}")
                psum2s[s] = psum2.tile([128, 2 * BLK], f32, tag="ps2",
                                       name=f"ps2_{s}")
                zsq = zpool.tile([128, 2, KS * KS], f32, tag="zsq",
                                 name=f"zsq{s}")
                szw = zpool.tile([128, 2, KS * KS], bf16, tag="szw",
                                 name=f"szw{s}")
                for kk in range(2):
                    nc.scalar.activation(zsq[:, kk, :], zts[s][:, kk, :],
                                         AF.Sqrt)
                    nc.vector.tensor_scalar_mul(
                        szw[:, kk, :], zsq[:, kk, :], w64[:, kk : kk + 1]
                    )
                planes[s] = [
                    plane.tile([128, 5 * HB], fp8, tag="pl0", name=f"pl0_{s}"),
                    plane.tile([128, 3 * HB], fp8, tag="pl1", name=f"pl1_{s}"),
                    plane.tile([64, 385], fp8, tag="pl7", name=f"pl7_{s}"),
                ]

                for pi, (b0, nbk) in enumerate(PIECES):
                    lo = b0 * BLK
                    ln = min(nbk * BLK, F - lo)
                    sxp = {}
                    for k in range(2):
                        t = sxq.tile([128, nbk * BLK], bf16, tag=f"sxp{pi}",
                                     name=f"sxp{s}{k}{pi}")
                        nc.scalar.activation(
                            t[:, :ln], xst[(s, k, pi)][:, :ln], AF.Sqrt
                        )
                        sxp[k] = t
                    for j in range(nbk):
                        b = b0 + j
                        if b < 7:
                            ps = psum.tile([128, HB], f32, tag="ps",
                                           name=f"ps_{s}_{b}")
                            for k in range(2):
                                for h in range(2):
                                    nc.tensor.matmul(
                                        ps[64 * h : 64 * h + 64, :],
                                        szw[:, k, :],
                                        sxp[k][:, j * BLK + h * HB
                                               : j * BLK + h * HB + HB],
                                        start=(k == 0),
                                        stop=(k == 1),
                                        tile_position=(0, 64 * h),
                                    )
                            if b <= 4:
                                nc.vector.tensor_copy(
                                    planes[s][0][:, HB * b : HB * b + HB],
                                    ps[:],
                                )
                            if 4 <= b <= 6:
                                nc.vector.tensor_copy(
                                    planes[s][1][:, HB * (b - 4)
                                                 : HB * (b - 4) + HB],
                                    ps[:],
                                )
                        else:
                            ps7 = psum7.tile([64, BLK], f32, tag="ps7",
                                             name=f"ps7_{s}")
                            for k in range(2):
                                nc.tensor.matmul(
                                    ps7[:, :385],
                                    szw[:, k, :],
                                    sxp[k][:, j * BLK : j * BLK + 385],
                                    start=(k == 0),
                                    stop=(k == 1),
                                )
                            nc.vector.tensor_copy(planes[s][2][:, 0:385],
                                                  ps7[:, :385])
                        if b == 4:
                            dump_sc0(s)
                        elif b == 6:
                            dump_sc1a(s)
                        elif b == 7:
                            dump_sc1b(s)

            # ---------------- stage 2 compute (tail) ----------------
            for s in range(SPC):
                ps2 = psum2s[s]
                for ch, (u0, ulen, sc, off, row) in enumerate(CHUNKS):
                    a = ats[(s, ch)]
                    cbase = 0 if ch != 3 else 96
                    m0 = 0
                    while m0 < ulen:
                        nb = min(BLK - (cbase + m0) % BLK, ulen - m0)
                        nc.tensor.matmul(
                            ps2[row : row + 1, cbase + m0 : cbase + m0 + nb],
                            ones[:],
                            a[:, m0 : m0 + nb],
                            start=True,
                            stop=True,
                            tile_position=(0, row),
                        )
                        m0 += nb
                nc.vector.tensor_copy(obufs[s][:, :], ps2[:, :])
                ob = obufs[s]
                osrc = bass.AP(ob[:].tensor, 0,
                               [[32 * 1024, 2], [MS, 16], [1, MO]])
                nc.sync.dma_start(out[s, 0, 0:32].unsqueeze(0), osrc)
                osrc = ob[64:65, 0 : 16 * MS].rearrange(
                    "p (i j) -> p i j", i=16
                )[:, :, 0:MO]
                nc.sync.dma_start(out[s, 0, 32:48].unsqueeze(0), osrc)
                osrc = ob[96:97, 0 : 8 * MS].rearrange(
                    "p (i j) -> p i j", i=8
                )[:, :, 0:MO]
                nc.sync.dma_start(out[s, 0, 48:MO].unsqueeze(0), osrc)

    nc.compile()
    return nc


def _get_nc():
    if "nc" not in _CACHE:
        _CACHE["nc"] = _build()
    return _CACHE["nc"]


def _run(z, x, weights, **runkw):
    z = np.ascontiguousarray(np.asarray(z), dtype=np.float32)
    x = np.ascontiguousarray(np.asarray(x), dtype=np.float32)
    w = np.ascontiguousarray(np.asarray(weights), dtype=np.float32).reshape(C)
    in_maps = []
    for i in range(NCORES):
        lo, hi = i * SPC, (i + 1) * SPC
        in_maps.append({"z": z[lo:hi], "x": x[lo:hi], "w": w})
    nc = _get_nc()
    try:
        res = run_bass_kernel_spmd(
            nc, in_maps, core_ids=list(range(NCORES)), **runkw
        )
    except Exception:
        # transient device errors have been observed to succeed on retry
        res = run_bass_kernel_spmd(
            nc, in_maps, core_ids=list(range(NCORES)), **runkw
        )
    full = np.concatenate([res.results[i]["out"] for i in range(NCORES)], axis=0)
    return full, res


def kernel(z, x, weights):
    full, _ = _run(z, x, weights)
    return full


# revision 13
# speedup vs baseline: 1.3736x; 1.2484x over previous
"""v3b reconstruction: best measured 50298 ns. Kept as fallback."""

import numpy as np

import concourse.bacc as bacc
import concourse.bass as bass
import concourse.mybir as mybir
from concourse import tile
from concourse.bass_utils import run_bass_kernel_spmd

N, C, KS, MS = 16, 256, 8, 63
MO = MS - KS + 1            # 56
F = MS * MS                 # 3969
NCORES = 8
SPC = N // NCORES           # samples per core
BLK = 512
W = (MO - 1) * MS + MO      # 3521
SH = 448
AF = mybir.ActivationFunctionType
f32 = mybir.dt.float32
bf16 = mybir.dt.bfloat16
fp8 = mybir.dt.float8e4

PIECES = [(0, 4), (4, 3), (7, 1)]
GCH = [(0, 1008), (1008, 2016), (2016, 3024), (3024, W)]
SCR = [(0, 2016 + SH), (2016, 3024 + SH), (3024, F)]
CHUNK_SC = [0, 0, 1, 2]
OUT_ROWS = {1: (0, 32), 3: (32, MO)}

_CACHE = {}


def _build():
    nc = bacc.Bacc("TRN2", target_bir_lowering=False, debug=False)
    z_in = nc.declare_dram_parameter("z", [SPC, C, KS, KS], f32, isOutput=False)
    x_in = nc.declare_dram_parameter("x", [SPC, C, MS, MS], f32, isOutput=False)
    w_in = nc.declare_dram_parameter("w", [C], f32, isOutput=False)
    out = nc.declare_dram_parameter("out", [SPC, 1, MO, MO], f32, isOutput=True)

    scs = [
        [nc.dram_tensor(f"sc{ci}_{s}", [64, c1 - c0], fp8)
         for ci, (c0, c1) in enumerate(SCR)]
        for s in range(SPC)
    ]

    xflat = x_in.rearrange("s (k c) h w -> s k c (h w)", c=128)

    with tile.TileContext(nc) as tc:
        with (
            tc.tile_pool(name="xstage", bufs=12) as xstage,
            tc.tile_pool(name="sxq", bufs=5) as sxq,
            tc.tile_pool(name="zpool", bufs=2) as zpool,
            tc.tile_pool(name="plane", bufs=2) as plane,
            tc.tile_pool(name="gath", bufs=4) as gath,
            tc.tile_pool(name="opool", bufs=1) as opool,
            tc.tile_pool(name="psum", bufs=4, space="PSUM") as psum,
            tc.tile_pool(name="psum2", bufs=2, space="PSUM") as psum2,
        ):
            wt = zpool.tile([128, 2], f32, name="wt")
            nc.sync.dma_start(wt[:], w_in.rearrange("(k c) -> c k", c=128))
            zts = []
            for s in range(SPC):
                zt = zpool.tile([128, 2, KS * KS], f32, tag="zt", name=f"zt{s}")
                nc.sync.dma_start(
                    zt[:], z_in[s].rearrange("(k c) p q -> c k (p q)", c=128)
                )
                zts.append(zt)
            xst = {}
            for s in range(SPC):
                for pi, (b0, nbk) in enumerate(PIECES):
                    for k in range(2):
                        lo = b0 * BLK
                        ln = min(nbk * BLK, F - lo)
                        t = xstage.tile([128, 4 * BLK], f32, tag="xst",
                                        name=f"xst{s}{k}{pi}")
                        nc.sync.dma_start(t[:, :ln], xflat[s, k, :, lo : lo + ln])
                        xst[(s, k, pi)] = t

            ones = opool.tile([64, 1], fp8, name="ones")
            nc.gpsimd.memset(ones[:], 1.0)
            w64 = zpool.tile([128, 2], f32, name="w64")
            nc.vector.tensor_scalar_mul(w64[:], wt[:], 1.0 / (KS * KS))

            obufs, psum2s, ats = {}, {}, {}
            for s in range(SPC):
                obuf = opool.tile([128, 1024], f32, tag=f"ob{s}",
                                  name=f"obuf{s}")
                obufs[s] = obuf
                psum2s[s] = psum2.tile([128, 2 * BLK], f32, tag="ps2",
                                       name=f"ps2_{s}")
                zsq = zpool.tile([128, 2, KS * KS], f32, tag="zsq", name=f"zsq{s}")
                szw = zpool.tile([128, 2, KS * KS], bf16, tag="szw", name=f"szw{s}")
                for kk in range(2):
                    nc.scalar.activation(zsq[:, kk, :], zts[s][:, kk, :], AF.Sqrt)
                    nc.vector.tensor_scalar_mul(
                        szw[:, kk, :], zsq[:, kk, :], w64[:, kk : kk + 1]
                    )

                pls = [
                    plane.tile([64, c1 - c0], fp8, tag=f"pl{ci}",
                               name=f"pl{s}_{ci}")
                    for ci, (c0, c1) in enumerate(SCR)
                ]
                evmap = [[] for _ in range(8)]
                for ci, (c0, c1) in enumerate(SCR):
                    for b in range(8):
                        lo = max(c0, b * BLK)
                        hi = min(c1, (b + 1) * BLK, F)
                        if lo < hi:
                            evmap[b].append((ci, lo - b * BLK, hi - b * BLK,
                                             lo - c0))
                last_block = [min((c1 - 1) // BLK, 7) for (c0, c1) in SCR]

                def emit_stage2(ci):
                    c0, c1 = SCR[ci]
                    pit = c1 - c0
                    nc.sync.dma_start(scs[s][ci][:, :], pls[ci][:])
                    for ch, (u0, u1) in enumerate(GCH):
                        if CHUNK_SC[ch] != ci:
                            continue
                        ulen = u1 - u0
                        a = gath.tile([64, 1008], fp8, tag="a",
                                      name=f"a{s}_{ch}")
                        src = bass.AP(
                            scs[s][ci][:].tensor,
                            u0 - c0,
                            [[8 * pit + MS, 8], [pit + 1, 8], [1, ulen]],
                        )
                        nc.gpsimd.dma_start(a[:, :ulen], src)
                        ats[(s, ch)] = a

                for pi, (b0, nbk) in enumerate(PIECES):
                    lo = b0 * BLK
                    ln = min(nbk * BLK, F - lo)
                    sxp = {}
                    for k in range(2):
                        t = sxq.tile([128, 4 * BLK], bf16, tag="sxp",
                                     name=f"sxp{s}{k}{pi}")
                        nc.scalar.activation(
                            t[:, :ln], xst[(s, k, pi)][:, :ln], AF.Sqrt
                        )
                        sxp[k] = t
                    for j in range(nbk):
                        b = b0 + j
                        nb = min(BLK, F - b * BLK)
                        ps = psum.tile([64, BLK], f32, tag="ps",
                                       name=f"ps_{s}_{b}")
                        for k in range(2):
                            nc.tensor.matmul(
                                ps[:, :nb],
                                szw[:, k, :],
                                sxp[k][:, j * BLK : j * BLK + nb],
                                start=(k == 0),
                                stop=(k == 1),
                            )
                        for (ci, p_lo, p_hi, d_lo) in evmap[b]:
                            nc.vector.tensor_copy(
                                pls[ci][:, d_lo : d_lo + (p_hi - p_lo)],
                                ps[:, p_lo:p_hi],
                            )
                        for ci in range(len(SCR)):
                            if last_block[ci] == b:
                                emit_stage2(ci)

            # ---- stage-2 compute, strictly after both samples' stage-1
            # so engine program order matches data readiness
            for s in range(SPC):
                ps2 = psum2s[s]
                for ch, (u0, u1) in enumerate(GCH):
                    ulen = u1 - u0
                    row = 32 * ch
                    a = ats[(s, ch)]
                    for m0 in range(0, ulen, BLK):
                        nb = min(BLK, ulen - m0)
                        nc.tensor.matmul(
                            ps2[row : row + 1, m0 : m0 + nb],
                            ones[:],
                            a[:, m0 : m0 + nb],
                            start=True,
                            stop=True,
                            tile_position=(0, row),
                        )
                nc.vector.tensor_copy(obufs[s][:, :], ps2[:, :])
                ob = obufs[s]
                osrc = bass.AP(ob[:].tensor, 0,
                               [[32 * 1024, 2], [MS, 16], [1, MO]])
                nc.sync.dma_start(out[s, 0, 0:32].unsqueeze(0), osrc)
                osrc = ob[64:65, 0 : 16 * MS].rearrange(
                    "p (i j) -> p i j", i=16
                )[:, :, 0:MO]
                nc.sync.dma_start(out[s, 0, 32:48].unsqueeze(0), osrc)
                osrc = ob[96:97, 0 : 8 * MS].rearrange(
                    "p (i j) -> p i j", i=8
                )[:, :, 0:MO]
                nc.sync.dma_start(out[s, 0, 48:MO].unsqueeze(0), osrc)

    nc.compile()
    return nc


def _get_nc():
    if "nc" not in _CACHE:
        _CACHE["nc"] = _build()
    return _CACHE["nc"]


def _run(z, x, weights, **runkw):
    z = np.ascontiguousarray(np.asarray(z), dtype=np.float32)
    x = np.ascontiguousarray(np.asarray(x), dtype=np.float32)
    w = np.ascontiguousarray(np.asarray(weights), dtype=np.float32).reshape(C)
    in_maps = []
    for i in range(NCORES):
        lo, hi = i * SPC, (i + 1) * SPC
        in_maps.append({"z": z[lo:hi], "x": x[lo:hi], "w": w})
    nc = _get_nc()
    try:
        res = run_bass_kernel_spmd(
            nc, in_maps, core_ids=list(range(NCORES)), **runkw
        )
    except Exception:
        res = run_bass_kernel_spmd(
            nc, in_maps, core_ids=list(range(NCORES)), **runkw
        )
    full = np.concatenate([res.results[i]["out"] for i in range(NCORES)], axis=0)
    return full, res


def kernel(z, x, weights):
    full, _ = _run(z, x, weights)
    return full
